# revision 5
# baseline (speedup 1.0000x reference)
"""CoAttention kernel for Trainium2, 8 NeuronCores, pure batch-data-parallel.

Math (per batch b):
  Q_proj = Q Wq^T + bq ; V_proj = V Wv^T + bv
  H = tanh(Q_proj V_proj^T) ; A_v = softmax(H, v-axis) ; A_q = softmax(H, q-axis)
  v_hat = sum_q (A_v V) ; q_hat = sum_v (A_q^T Q)

Only the summed outputs are needed, so with E = exp(tanh(S)) (S bounded in
[-1,1] after tanh -> exp never overflows, no max-subtraction needed):
  s_q[q] = sum_v E ; s_v[v] = sum_q E ; r_* = 1/s_*
  v_hat = r_q^T (E^T^T V)   (B-matmul form)
  q_hat = (E^T r_v)^T Q
and S = Q M0 V^T + (Q u) 1^T + 1 (V z)^T + c with
  M0 = Wq^T Wv, u = Wq^T bv, z = Wv^T bq, c = bq.bv
computed on-chip; rank-1 terms folded into the S^T matmul (K=1 group) and
the tanh bias (per-partition).
"""

import numpy as np
from contextlib import ExitStack

import concourse.bacc as bacc
import concourse.mybir as mybir
from concourse import tile
from concourse.bass_utils import run_bass_kernel_spmd
from concourse.kernels.tile_matmul import make_identity

F32 = mybir.dt.float32
F32R = mybir.dt.float32r
BF16 = mybir.dt.bfloat16
TANH = mybir.ActivationFunctionType.Tanh
EXP = mybir.ActivationFunctionType.Exp

N_CORES = 8
B, NV, NQ = 64, 1024, 512
QD, VD, HID = 768, 512, 512
BL = B // N_CORES  # batches per core

P = 128
NQT = NQ // P   # 4 q tiles
NVT = NV // P   # 8 v tiles
NDT = QD // P   # 6 d tiles
ND2T = VD // P  # 4 d2 tiles
NHT = HID // P  # 4 h tiles


def _build_program():
    nc = bacc.Bacc("TRN2", target_bir_lowering=False, debug=False,
                   num_devices=N_CORES)

    Qd = nc.dram_tensor("Qd", [BL, NQ, QD], F32, kind="ExternalInput")
    Vd = nc.dram_tensor("Vd", [BL, NV, VD], F32, kind="ExternalInput")
    Wq = nc.dram_tensor("Wq", [HID, QD], F32, kind="ExternalInput")
    Wqb = nc.dram_tensor("Wqb", [HID], F32, kind="ExternalInput")
    Wv = nc.dram_tensor("Wv", [HID, VD], F32, kind="ExternalInput")
    Wvb = nc.dram_tensor("Wvb", [HID], F32, kind="ExternalInput")
    vh = nc.dram_tensor("vh", [BL, VD], F32, kind="ExternalOutput")
    qh = nc.dram_tensor("qh", [BL, QD], F32, kind="ExternalOutput")

    with tile.TileContext(nc) as tc, ExitStack() as ctx:
        const = ctx.enter_context(tc.tile_pool(name="const", bufs=1))

        # ---- persistent constants ----
        ident = const.tile([P, P], BF16)
        make_identity(nc, ident[:])
        ones_row = const.tile([1, P], BF16)
        nc.any.memset(ones_row[:], 1.0)
        m0_bf = const.tile([P, NDT, VD], BF16)      # M0 [d, d2]
        u_bf = const.tile([P, NDT, 1], BF16)        # u col [d, 1]
        z_bf = const.tile([P, ND2T, 1], BF16)       # z col [d2, 1]
        c_bc = const.tile([P, 1], F32)              # c broadcast to 128 parts

        # ---- setup: weights -> M0_ext, u, z, c ----
        with tc.tile_pool(name="setup", bufs=1) as su, \
             tc.tile_pool(name="setup_ps", bufs=2, space="PSUM") as sps:
            wq_sb = su.tile([P, NHT, QD], F32)
            nc.sync.dma_start(out=wq_sb[:],
                              in_=Wq.ap().rearrange("(t p) d -> p t d", p=P))
            wv_sb = su.tile([P, NHT, VD], F32)
            nc.sync.dma_start(out=wv_sb[:],
                              in_=Wv.ap().rearrange("(t p) d -> p t d", p=P))
            bq_sb = su.tile([P, NHT], F32)
            nc.sync.dma_start(out=bq_sb[:],
                              in_=Wqb.ap().rearrange("(t p) -> p t", p=P))
            bv_sb = su.tile([P, NHT], F32)
            nc.sync.dma_start(out=bv_sb[:],
                              in_=Wvb.ap().rearrange("(t p) -> p t", p=P))

            # M0[d, d2] = sum_h Wq[h, d] Wv[h, d2]; u[d] = sum_h Wq[h, d] bv[h]
            for m in range(NDT):
                m0_ps = sps.tile([P, VD], F32, tag="m0")
                u_ps = sps.tile([P, 1], F32, tag="u")
                for k in range(NHT):
                    lhs = wq_sb[:, k, m * P:(m + 1) * P]
                    nc.tensor.matmul(m0_ps[:], lhs, wv_sb[:, k, :],
                                     start=(k == 0), stop=(k == NHT - 1))
                    nc.tensor.matmul(u_ps[:], lhs, bv_sb[:, k:k + 1],
                                     start=(k == 0), stop=(k == NHT - 1))
                nc.vector.tensor_copy(m0_bf[:, m, :], m0_ps[:])
                nc.vector.tensor_copy(u_bf[:, m, :], u_ps[:])

            # z[d2] = sum_h Wv[h, d2] bq[h] ; c = bq . bv
            for m in range(ND2T):
                z_ps = sps.tile([P, 1], F32, tag="u")
                for k in range(NHT):
                    nc.tensor.matmul(z_ps[:], wv_sb[:, k, m * P:(m + 1) * P],
                                     bq_sb[:, k:k + 1],
                                     start=(k == 0), stop=(k == NHT - 1))
                nc.vector.tensor_copy(z_bf[:, m, :], z_ps[:])
            c_ps = sps.tile([1, 1], F32, tag="u")
            for k in range(NHT):
                nc.tensor.matmul(c_ps[:], bq_sb[:, k:k + 1], bv_sb[:, k:k + 1],
                                 start=(k == 0), stop=(k == NHT - 1))
            c_sb = su.tile([1, 1], F32)
            nc.vector.tensor_copy(c_sb[:], c_ps[:])
            nc.gpsimd.partition_broadcast(c_bc[:], c_sb[:])

        # ---- streaming pools ----
        pbf = ctx.enter_context(tc.tile_pool(name="pbf", bufs=2))
        ptr = ctx.enter_context(tc.tile_pool(name="ptr", bufs=2))
        pmid = ctx.enter_context(tc.tile_pool(name="pmid", bufs=2))
        pvec = ctx.enter_context(tc.tile_pool(name="pvec", bufs=2))

        ps_tp = ctx.enter_context(tc.tile_pool(name="ps_tp", bufs=1, space="PSUM"))
        ps_g = ctx.enter_context(tc.tile_pool(name="ps_g", bufs=2, space="PSUM"))
        ps_s = ctx.enter_context(tc.tile_pool(name="ps_s", bufs=2, space="PSUM"))
        ps_b = ctx.enter_context(tc.tile_pool(name="ps_b", bufs=1, space="PSUM"))
        ps_row = ctx.enter_context(tc.tile_pool(name="ps_row", bufs=1, space="PSUM"))
        ps_col = ctx.enter_context(tc.tile_pool(name="ps_col", bufs=1, space="PSUM"))

        for b in range(BL):
            # -- load + cast to bf16 in one SWDGE DMA --
            qb16 = pbf.tile([P, NQT, QD], BF16, tag="qb16")
            nc.gpsimd.dma_start(out=qb16[:],
                                in_=Qd.ap()[b].rearrange("(t p) d -> p t d", p=P))
            vb16 = pbf.tile([P, NVT, VD], BF16, tag="vb16")
            nc.gpsimd.dma_start(out=vb16[:],
                                in_=Vd.ap()[b].rearrange("(t p) d -> p t d", p=P))

            # -- PE transposes -> Q^T [d, q], V^T [d2, v] --
            qt_bf = ptr.tile([P, NDT, NQ], BF16, tag="qt")
            for kd in range(NDT):
                tp = ps_tp.tile([P, NQ], BF16, tag="tp")
                for tq in range(NQT):
                    nc.tensor.transpose(tp[:, tq * P:(tq + 1) * P],
                                        qb16[:, tq, kd * P:(kd + 1) * P],
                                        ident[:])
                nc.vector.tensor_copy(qt_bf[:, kd, :], tp[:])
            vt_bf = ptr.tile([P, ND2T, NV], BF16, tag="vt")
            for kd in range(ND2T):
                tp = ps_tp.tile([P, NV], BF16, tag="tp")
                for tv in range(NVT):
                    nc.tensor.transpose(tp[:, tv * P:(tv + 1) * P],
                                        vb16[:, tv, kd * P:(kd + 1) * P],
                                        ident[:])
                nc.vector.tensor_copy(vt_bf[:, kd, :], tp[:])

            # -- G~^T [d2, q] = M0^T Q^T ; Qu row --
            gt_bf = pmid.tile([P, ND2T, NQ], BF16, tag="gt")
            for m in range(ND2T):
                g_ps = ps_g.tile([P, NQ], F32, tag="g")
                for k in range(NDT):
                    nc.tensor.matmul(g_ps[:], m0_bf[:, k, m * P:(m + 1) * P],
                                     qt_bf[:, k, :],
                                     start=(k == 0), stop=(k == NDT - 1))
                nc.vector.tensor_copy(gt_bf[:, m, :], g_ps[:])
            qu_ps = ps_row.tile([1, NQ], F32, tag="row512")
            for k in range(NDT):
                nc.tensor.matmul(qu_ps[:], u_bf[:, k, :], qt_bf[:, k, :],
                                 start=(k == 0), stop=(k == NDT - 1))
            quc_row = pvec.tile([1, NQ], BF16, tag="quc")
            nc.vector.tensor_copy(quc_row[:], qu_ps[:])

            # -- S^T [v, q] (+ Qu K=1 fold) and Vz col --
            vz_ps = ps_col.tile([P, NVT], F32, tag="col")
            s_ps_list = []
            t_bf = pmid.tile([P, NVT, NQ], BF16, tag="tbf")
            e_bf = pmid.tile([P, NVT, NQ], BF16, tag="ebf")
            sv_f = pvec.tile([P, NVT], F32, tag="sv")
            vzc = pvec.tile([P, NVT], F32, tag="vzc")
            for m in range(NVT):
                s_ps = ps_s.tile([P, NQ], F32, tag="s")
                for k in range(ND2T):
                    lhs = vt_bf[:, k, m * P:(m + 1) * P]
                    nc.tensor.matmul(s_ps[:], lhs, gt_bf[:, k, :],
                                     start=(k == 0), stop=False)
                    nc.tensor.matmul(vz_ps[:, m:m + 1], lhs, z_bf[:, k, :],
                                     start=(k == 0), stop=(k == ND2T - 1))
                nc.tensor.matmul(s_ps[:], ones_row[:], quc_row[:],
                                 start=False, stop=True)
                # tanh bias needs vzc ready; compute once after m==0 kicks vz
                if m == NVT - 1:
                    nc.vector.tensor_scalar_add(vzc[:], vz_ps[:], c_bc[:])
                s_ps_list.append(s_ps)

            # -- tanh (bias = Vz + c per v-partition) then exp (+ s_v accum) --
            for m in range(NVT):
                nc.scalar.activation(t_bf[:, m, :], s_ps_list[m][:], TANH,
                                     bias=vzc[:, m:m + 1])
            for m in range(NVT):
                nc.scalar.activation(e_bf[:, m, :], t_bf[:, m, :], EXP,
                                     accum_out=sv_f[:, m:m + 1])

            rv_f = pvec.tile([P, NVT], F32, tag="rvf")
            nc.vector.reciprocal(rv_f[:], sv_f[:])
            rr_bf = pvec.tile([P, NVT, 2], BF16, tag="rr")
            nc.any.memset(rr_bf[:, :, 0:1], 1.0)
            nc.vector.tensor_copy(rr_bf[:, :, 1:2], rv_f[:].unsqueeze(-1))

            # -- stationary-E^T group: [s_q | c_q] and B = E^T^T V --
            sc_ps = ps_col.tile([P, 2 * NQT], F32, tag="col")
            bb_bf = pmid.tile([P, NQT, VD], BF16, tag="bb")
            for m in range(NQT):
                b_ps = ps_b.tile([P, VD], F32, tag="b")
                for k in range(NVT):
                    lhs = e_bf[:, k, m * P:(m + 1) * P]
                    nc.tensor.matmul(sc_ps[:, 2 * m:2 * m + 2], lhs,
                                     rr_bf[:, k, :],
                                     start=(k == 0), stop=(k == NVT - 1))
                    nc.tensor.matmul(b_ps[:], lhs, vb16[:, k, :],
                                     start=(k == 0), stop=(k == NVT - 1))
                nc.vector.tensor_copy(bb_bf[:, m, :], b_ps[:])

            rq_f = pvec.tile([P, NQT], F32, tag="rqf")
            nc.vector.reciprocal(rq_f[:], sc_ps[:, 0:2 * NQT:2])
            rq_bf = pvec.tile([P, NQT], BF16, tag="rqb")
            nc.vector.tensor_copy(rq_bf[:], rq_f[:])
            cq_bf = pvec.tile([P, NQT], BF16, tag="cqb")
            nc.vector.tensor_copy(cq_bf[:], sc_ps[:, 1:2 * NQT:2])

            # -- v_hat = r_q^T B ; q_hat = c_q^T Q --
            vh_ps = ps_row.tile([1, VD], F32, tag="row512")
            for k in range(NQT):
                nc.tensor.matmul(vh_ps[:], rq_bf[:, k:k + 1], bb_bf[:, k, :],
                                 start=(k == 0), stop=(k == NQT - 1))
            ov_sb = pvec.tile([1, VD], F32, tag="ov")
            nc.vector.tensor_copy(ov_sb[:], vh_ps[:])
            nc.sync.dma_start(out=vh.ap()[b:b + 1, :], in_=ov_sb[:])
            qh_ps1 = ps_row.tile([1, 512], F32, tag="row512")
            qh_ps2 = ps_row.tile([1, QD - 512], F32, tag="row512")
            for k in range(NQT):
                nc.tensor.matmul(qh_ps1[:], cq_bf[:, k:k + 1],
                                 qb16[:, k, 0:512],
                                 start=(k == 0), stop=(k == NQT - 1))
                nc.tensor.matmul(qh_ps2[:], cq_bf[:, k:k + 1],
                                 qb16[:, k, 512:QD],
                                 start=(k == 0), stop=(k == NQT - 1))
            oq_sb = pvec.tile([1, QD], F32, tag="oq")
            nc.vector.tensor_copy(oq_sb[:, 0:512], qh_ps1[:])
            nc.vector.tensor_copy(oq_sb[:, 512:QD], qh_ps2[:])
            nc.sync.dma_start(out=qh.ap()[b:b + 1, :], in_=oq_sb[:])

    nc.compile()
    return nc


_NC_CACHE = None


def _get_program():
    global _NC_CACHE
    if _NC_CACHE is None:
        _NC_CACHE = _build_program()
    return _NC_CACHE


def kernel(V, Q, Wq_w, Wq_b, Wv_w, Wv_b):
    V = np.ascontiguousarray(np.asarray(V, dtype=np.float32))
    Q = np.ascontiguousarray(np.asarray(Q, dtype=np.float32))
    Wq_w = np.ascontiguousarray(np.asarray(Wq_w, dtype=np.float32))
    Wq_b = np.ascontiguousarray(np.asarray(Wq_b, dtype=np.float32))
    Wv_w = np.ascontiguousarray(np.asarray(Wv_w, dtype=np.float32))
    Wv_b = np.ascontiguousarray(np.asarray(Wv_b, dtype=np.float32))

    nc = _get_program()
    in_maps = []
    for i in range(N_CORES):
        sl = slice(i * BL, (i + 1) * BL)
        in_maps.append({
            "Qd": np.ascontiguousarray(Q[sl]),
            "Vd": np.ascontiguousarray(V[sl]),
            "Wq": Wq_w, "Wqb": Wq_b, "Wv": Wv_w, "Wvb": Wv_b,
        })
    res = run_bass_kernel_spmd(nc, in_maps, list(range(N_CORES)))
    v_hat = np.concatenate([res.results[i]["vh"] for i in range(N_CORES)], 0)
    q_hat = np.concatenate([res.results[i]["qh"] for i in range(N_CORES)], 0)
    return (v_hat.astype(np.float32), q_hat.astype(np.float32))


# revision 6
# speedup vs baseline: 1.3417x; 1.3417x over previous
"""CoAttention kernel for Trainium2, 8 NeuronCores, pure batch-data-parallel.

Math (per batch b):
  Q_proj = Q Wq^T + bq ; V_proj = V Wv^T + bv
  H = tanh(Q_proj V_proj^T) ; A_v = softmax(H, v-axis) ; A_q = softmax(H, q-axis)
  v_hat = sum_q (A_v V) ; q_hat = sum_v (A_q^T Q)

Only the summed outputs are needed, so with E = exp(tanh(S)) (S bounded in
[-1,1] after tanh -> exp never overflows, no max-subtraction needed):
  s_q[q] = sum_v E ; s_v[v] = sum_q E ; r_* = 1/s_*
  v_hat = r_q^T (E^T^T V)   (B-matmul form)
  q_hat = (E^T r_v)^T Q
and S = Q M0 V^T + (Q u) 1^T + 1 (V z)^T + c with
  M0 = Wq^T Wv, u = Wq^T bv, z = Wv^T bq, c = bq.bv
computed on-chip; rank-1 terms folded into the S^T matmul (K=1 group) and
the tanh bias (per-partition).
"""

import numpy as np
from contextlib import ExitStack

import concourse.bacc as bacc
import concourse.mybir as mybir
from concourse import tile
from concourse.bass_utils import run_bass_kernel_spmd
from concourse.kernels.tile_matmul import make_identity

F32 = mybir.dt.float32
F32R = mybir.dt.float32r
BF16 = mybir.dt.bfloat16
TANH = mybir.ActivationFunctionType.Tanh
EXP = mybir.ActivationFunctionType.Exp

N_CORES = 8
B, NV, NQ = 64, 1024, 512
QD, VD, HID = 768, 512, 512
BL = B // N_CORES  # batches per core

P = 128
NQT = NQ // P   # 4 q tiles
NVT = NV // P   # 8 v tiles
NDT = QD // P   # 6 d tiles
ND2T = VD // P  # 4 d2 tiles
NHT = HID // P  # 4 h tiles


def _build_program():
    nc = bacc.Bacc("TRN2", target_bir_lowering=False, debug=False,
                   num_devices=N_CORES)

    Qd = nc.dram_tensor("Qd", [BL, NQ, QD], F32, kind="ExternalInput")
    Vd = nc.dram_tensor("Vd", [BL, NV, VD], F32, kind="ExternalInput")
    Wq = nc.dram_tensor("Wq", [HID, QD], F32, kind="ExternalInput")
    Wqb = nc.dram_tensor("Wqb", [HID], F32, kind="ExternalInput")
    Wv = nc.dram_tensor("Wv", [HID, VD], F32, kind="ExternalInput")
    Wvb = nc.dram_tensor("Wvb", [HID], F32, kind="ExternalInput")
    vh = nc.dram_tensor("vh", [BL, VD], F32, kind="ExternalOutput")
    qh = nc.dram_tensor("qh", [BL, QD], F32, kind="ExternalOutput")

    with tile.TileContext(nc) as tc, ExitStack() as ctx:
        const = ctx.enter_context(tc.tile_pool(name="const", bufs=1))

        # ---- persistent constants ----
        ident = const.tile([P, P], BF16)
        make_identity(nc, ident[:])
        ones_row = const.tile([1, P], BF16)
        nc.any.memset(ones_row[:], 1.0)
        m0_bf = const.tile([P, NDT, VD], BF16)      # M0 [d, d2]
        u_bf = const.tile([P, NDT, 1], BF16)        # u col [d, 1]
        z_bf = const.tile([P, ND2T, 1], BF16)       # z col [d2, 1]
        c_bc = const.tile([P, 1], F32)              # c broadcast to 128 parts

        # ---- setup: weights -> M0_ext, u, z, c ----
        with tc.tile_pool(name="setup", bufs=1) as su, \
             tc.tile_pool(name="setup_ps", bufs=2, space="PSUM") as sps:
            wq_sb = su.tile([P, NHT, QD], F32)
            nc.sync.dma_start(out=wq_sb[:],
                              in_=Wq.ap().rearrange("(t p) d -> p t d", p=P))
            wv_sb = su.tile([P, NHT, VD], F32)
            nc.sync.dma_start(out=wv_sb[:],
                              in_=Wv.ap().rearrange("(t p) d -> p t d", p=P))
            bq_sb = su.tile([P, NHT], F32)
            nc.sync.dma_start(out=bq_sb[:],
                              in_=Wqb.ap().rearrange("(t p) -> p t", p=P))
            bv_sb = su.tile([P, NHT], F32)
            nc.sync.dma_start(out=bv_sb[:],
                              in_=Wvb.ap().rearrange("(t p) -> p t", p=P))

            # M0[d, d2] = sum_h Wq[h, d] Wv[h, d2]; u[d] = sum_h Wq[h, d] bv[h]
            for m in range(NDT):
                m0_ps = sps.tile([P, VD], F32, tag="m0")
                u_ps = sps.tile([P, 1], F32, tag="u")
                for k in range(NHT):
                    lhs = wq_sb[:, k, m * P:(m + 1) * P]
                    nc.tensor.matmul(m0_ps[:], lhs, wv_sb[:, k, :],
                                     start=(k == 0), stop=(k == NHT - 1))
                    nc.tensor.matmul(u_ps[:], lhs, bv_sb[:, k:k + 1],
                                     start=(k == 0), stop=(k == NHT - 1))
                nc.vector.tensor_copy(m0_bf[:, m, :], m0_ps[:])
                nc.vector.tensor_copy(u_bf[:, m, :], u_ps[:])

            # z[d2] = sum_h Wv[h, d2] bq[h] ; c = bq . bv
            for m in range(ND2T):
                z_ps = sps.tile([P, 1], F32, tag="u")
                for k in range(NHT):
                    nc.tensor.matmul(z_ps[:], wv_sb[:, k, m * P:(m + 1) * P],
                                     bq_sb[:, k:k + 1],
                                     start=(k == 0), stop=(k == NHT - 1))
                nc.vector.tensor_copy(z_bf[:, m, :], z_ps[:])
            c_ps = sps.tile([1, 1], F32, tag="u")
            for k in range(NHT):
                nc.tensor.matmul(c_ps[:], bq_sb[:, k:k + 1], bv_sb[:, k:k + 1],
                                 start=(k == 0), stop=(k == NHT - 1))
            c_sb = su.tile([1, 1], F32)
            nc.vector.tensor_copy(c_sb[:], c_ps[:])
            nc.gpsimd.partition_broadcast(c_bc[:], c_sb[:])

        # ---- streaming pools ----
        pbf = ctx.enter_context(tc.tile_pool(name="pbf", bufs=2))
        ptr = ctx.enter_context(tc.tile_pool(name="ptr", bufs=2))
        pmid = ctx.enter_context(tc.tile_pool(name="pmid", bufs=2))
        pvec = ctx.enter_context(tc.tile_pool(name="pvec", bufs=2))

        ps_tp = ctx.enter_context(tc.tile_pool(name="ps_tp", bufs=1, space="PSUM"))
        ps_g = ctx.enter_context(tc.tile_pool(name="ps_g", bufs=2, space="PSUM"))
        ps_s = ctx.enter_context(tc.tile_pool(name="ps_s", bufs=2, space="PSUM"))
        ps_b = ctx.enter_context(tc.tile_pool(name="ps_b", bufs=1, space="PSUM"))
        ps_row = ctx.enter_context(tc.tile_pool(name="ps_row", bufs=1, space="PSUM"))
        ps_col = ctx.enter_context(tc.tile_pool(name="ps_col", bufs=1, space="PSUM"))

        for b in range(BL):
            # -- load + cast to bf16 in one SWDGE DMA --
            qb16 = pbf.tile([P, NQT, QD], BF16, tag="qb16")
            nc.gpsimd.dma_start(out=qb16[:],
                                in_=Qd.ap()[b].rearrange("(t p) d -> p t d", p=P))
            vb16 = pbf.tile([P, NVT, VD], BF16, tag="vb16")
            nc.gpsimd.dma_start(out=vb16[:],
                                in_=Vd.ap()[b].rearrange("(t p) d -> p t d", p=P))

            # -- PE transposes -> Q^T [d, q], V^T [d2, v] --
            qt_bf = ptr.tile([P, NDT, NQ], BF16, tag="qt")
            for kd in range(NDT):
                tp = ps_tp.tile([P, NQ], BF16, tag="tp")
                for tq in range(NQT):
                    nc.tensor.transpose(tp[:, tq * P:(tq + 1) * P],
                                        qb16[:, tq, kd * P:(kd + 1) * P],
                                        ident[:])
                nc.vector.tensor_copy(qt_bf[:, kd, :], tp[:])
            vt_bf = ptr.tile([P, ND2T, NV], BF16, tag="vt")
            for kd in range(ND2T):
                tp = ps_tp.tile([P, NV], BF16, tag="tp")
                for tv in range(NVT):
                    nc.tensor.transpose(tp[:, tv * P:(tv + 1) * P],
                                        vb16[:, tv, kd * P:(kd + 1) * P],
                                        ident[:])
                nc.vector.tensor_copy(vt_bf[:, kd, :], tp[:])

            # -- G~^T [d2, q] = M0^T Q^T ; Qu row --
            gt_bf = pmid.tile([P, ND2T, NQ], BF16, tag="gt")
            for m in range(ND2T):
                g_ps = ps_g.tile([P, NQ], F32, tag="g")
                for k in range(NDT):
                    nc.tensor.matmul(g_ps[:], m0_bf[:, k, m * P:(m + 1) * P],
                                     qt_bf[:, k, :],
                                     start=(k == 0), stop=(k == NDT - 1))
                nc.vector.tensor_copy(gt_bf[:, m, :], g_ps[:])
            qu_ps = ps_row.tile([1, NQ], F32, tag="row512")
            for k in range(NDT):
                nc.tensor.matmul(qu_ps[:], u_bf[:, k, :], qt_bf[:, k, :],
                                 start=(k == 0), stop=(k == NDT - 1))
            quc_row = pvec.tile([1, NQ], BF16, tag="quc")
            nc.vector.tensor_copy(quc_row[:], qu_ps[:])

            # -- S^T [v, q] (+ Qu K=1 fold) and Vz col --
            vz_ps = ps_col.tile([P, NVT], F32, tag="col")
            s_ps_list = []
            t_bf = pmid.tile([P, NVT, NQ], BF16, tag="tbf")
            e_bf = pmid.tile([P, NVT, NQ], BF16, tag="ebf")
            sv_f = pvec.tile([P, NVT], F32, tag="sv")
            vzc = pvec.tile([P, NVT], F32, tag="vzc")
            for m in range(NVT):
                s_ps = ps_s.tile([P, NQ], F32, tag="s")
                for k in range(ND2T):
                    lhs = vt_bf[:, k, m * P:(m + 1) * P]
                    nc.tensor.matmul(s_ps[:], lhs, gt_bf[:, k, :],
                                     start=(k == 0), stop=False)
                    nc.tensor.matmul(vz_ps[:, m:m + 1], lhs, z_bf[:, k, :],
                                     start=(k == 0), stop=(k == ND2T - 1))
                nc.tensor.matmul(s_ps[:], ones_row[:], quc_row[:],
                                 start=False, stop=True)
                # tanh bias needs vzc ready; compute once after m==0 kicks vz
                if m == NVT - 1:
                    nc.vector.tensor_scalar_add(vzc[:], vz_ps[:], c_bc[:])
                s_ps_list.append(s_ps)

            # -- tanh (bias = Vz + c per v-partition) then exp (+ s_v accum) --
            for m in range(NVT):
                nc.scalar.activation(t_bf[:, m, :], s_ps_list[m][:], TANH,
                                     bias=vzc[:, m:m + 1])
            for m in range(NVT):
                nc.scalar.activation(e_bf[:, m, :], t_bf[:, m, :], EXP,
                                     accum_out=sv_f[:, m:m + 1])

            rv_f = pvec.tile([P, NVT], F32, tag="rvf")
            nc.vector.reciprocal(rv_f[:], sv_f[:])
            rr_bf = pvec.tile([P, NVT, 2], BF16, tag="rr")
            nc.any.memset(rr_bf[:, :, 0:1], 1.0)
            nc.vector.tensor_copy(rr_bf[:, :, 1:2], rv_f[:].unsqueeze(-1))

            # -- stationary-E^T group: [s_q | c_q] and B = E^T^T V --
            sc_ps = ps_col.tile([P, 2 * NQT], F32, tag="col")
            bb_bf = pmid.tile([P, NQT, VD], BF16, tag="bb")
            for m in range(NQT):
                b_ps = ps_b.tile([P, VD], F32, tag="b")
                for k in range(NVT):
                    lhs = e_bf[:, k, m * P:(m + 1) * P]
                    nc.tensor.matmul(sc_ps[:, 2 * m:2 * m + 2], lhs,
                                     rr_bf[:, k, :],
                                     start=(k == 0), stop=(k == NVT - 1))
                    nc.tensor.matmul(b_ps[:], lhs, vb16[:, k, :],
                                     start=(k == 0), stop=(k == NVT - 1))
                nc.vector.tensor_copy(bb_bf[:, m, :], b_ps[:])

            rq_f = pvec.tile([P, NQT], F32, tag="rqf")
            nc.vector.reciprocal(rq_f[:], sc_ps[:, 0:2 * NQT:2])
            rq_bf = pvec.tile([P, NQT], BF16, tag="rqb")
            nc.vector.tensor_copy(rq_bf[:], rq_f[:])
            cq_bf = pvec.tile([P, NQT], BF16, tag="cqb")
            nc.vector.tensor_copy(cq_bf[:], sc_ps[:, 1:2 * NQT:2])

            # -- v_hat = r_q^T B ; q_hat = c_q^T Q --
            vh_ps = ps_row.tile([1, VD], F32, tag="row512")
            for k in range(NQT):
                nc.tensor.matmul(vh_ps[:], rq_bf[:, k:k + 1], bb_bf[:, k, :],
                                 start=(k == 0), stop=(k == NQT - 1))
            ov_sb = pvec.tile([1, VD], F32, tag="ov")
            nc.vector.tensor_copy(ov_sb[:], vh_ps[:])
            nc.sync.dma_start(out=vh.ap()[b:b + 1, :], in_=ov_sb[:])
            qh_ps1 = ps_row.tile([1, 512], F32, tag="row512")
            qh_ps2 = ps_row.tile([1, QD - 512], F32, tag="row512")
            for k in range(NQT):
                nc.tensor.matmul(qh_ps1[:], cq_bf[:, k:k + 1],
                                 qb16[:, k, 0:512],
                                 start=(k == 0), stop=(k == NQT - 1))
                nc.tensor.matmul(qh_ps2[:], cq_bf[:, k:k + 1],
                                 qb16[:, k, 512:QD],
                                 start=(k == 0), stop=(k == NQT - 1))
            oq_sb = pvec.tile([1, QD], F32, tag="oq")
            nc.vector.tensor_copy(oq_sb[:, 0:512], qh_ps1[:])
            nc.vector.tensor_copy(oq_sb[:, 512:QD], qh_ps2[:])
            nc.sync.dma_start(out=qh.ap()[b:b + 1, :], in_=oq_sb[:])

    nc.compile()
    return nc


_NC_CACHE = None
_RUNNER_CACHE = None


def _get_program():
    global _NC_CACHE
    if _NC_CACHE is None:
        _NC_CACHE = _build_program()
    return _NC_CACHE


def _make_runner():
    """Build the sharded PJRT callable once (mirrors
    concourse.bass2jax.run_bass_via_pjrt, but caches the jitted function so
    repeat kernel() calls skip retrace/recompile)."""
    import jax
    from jax.sharding import Mesh, PartitionSpec
    from jax.experimental.shard_map import shard_map
    from concourse import bass2jax
    from concourse import mybir as _mybir

    nc = _get_program()
    bass2jax.install_neuronx_cc_hook()

    partition_name = (nc.partition_id_tensor.name
                      if nc.partition_id_tensor else None)
    in_names, out_names, out_avals, zero_shapes = [], [], [], []
    for alloc in nc.m.functions[0].allocations:
        if not isinstance(alloc, _mybir.MemoryLocationSet):
            continue
        name = alloc.memorylocations[0].name
        if alloc.kind == "ExternalInput":
            if name != partition_name:
                in_names.append(name)
        elif alloc.kind == "ExternalOutput":
            out_names.append(name)
            shape = tuple(alloc.tensor_shape)
            dtype = _mybir.dt.np(alloc.dtype)
            out_avals.append(jax.core.ShapedArray(shape, dtype))
            zero_shapes.append((shape, dtype))
    n_params = len(in_names)
    n_outs = len(out_avals)
    param_names = list(in_names)
    in_names = in_names + out_names
    if partition_name is not None:
        in_names.append(partition_name)
    donate = tuple(range(n_params, n_params + n_outs))

    def _body(*args):
        operands = list(args)
        if partition_name is not None:
            operands.append(bass2jax.partition_id_tensor())
        outs = bass2jax._bass_exec_p.bind(
            *operands,
            out_avals=tuple(out_avals),
            in_names=tuple(in_names),
            out_names=tuple(out_names),
            lowering_input_output_aliases=(),
            sim_require_finite=True,
            sim_require_nnan=True,
            nc=nc,
        )
        return tuple(outs)

    devices = jax.devices()[:N_CORES]
    mesh = Mesh(np.asarray(devices), ("core",))
    in_specs = (PartitionSpec("core"),) * (n_params + n_outs)
    out_specs = (PartitionSpec("core"),) * len(out_names)
    sharded = jax.jit(
        shard_map(_body, mesh=mesh, in_specs=in_specs, out_specs=out_specs,
                  check_rep=False),
        donate_argnums=donate, keep_unused=True)

    def run(global_ins: dict):
        concat_in = [global_ins[n] for n in param_names]
        concat_zeros = [np.zeros((N_CORES * s[0], *s[1:]), d)
                        for (s, d) in zero_shapes]
        out_arrs = sharded(*concat_in, *concat_zeros)
        return {name: np.asarray(out_arrs[i])
                for i, name in enumerate(out_names)}

    return run


def kernel(V, Q, Wq_w, Wq_b, Wv_w, Wv_b):
    global _RUNNER_CACHE
    V = np.ascontiguousarray(np.asarray(V, dtype=np.float32))
    Q = np.ascontiguousarray(np.asarray(Q, dtype=np.float32))
    Wq_w = np.ascontiguousarray(np.asarray(Wq_w, dtype=np.float32))
    Wq_b = np.ascontiguousarray(np.asarray(Wq_b, dtype=np.float32))
    Wv_w = np.ascontiguousarray(np.asarray(Wv_w, dtype=np.float32))
    Wv_b = np.ascontiguousarray(np.asarray(Wv_b, dtype=np.float32))

    if _RUNNER_CACHE is None:
        _RUNNER_CACHE = _make_runner()

    # shard_map global arrays: per-core slices concat on axis 0. Q/V are
    # already exactly that; weights get tiled 8x.
    tile8 = lambda a: np.concatenate([a] * N_CORES, axis=0)
    outs = _RUNNER_CACHE({
        "Qd": Q, "Vd": V,
        "Wq": tile8(Wq_w), "Wqb": tile8(Wq_b),
        "Wv": tile8(Wv_w), "Wvb": tile8(Wv_b),
    })
    v_hat = outs["vh"].reshape(B, VD)
    q_hat = outs["qh"].reshape(B, QD)
    return (v_hat.astype(np.float32), q_hat.astype(np.float32))
